# revision 23
# baseline (speedup 1.0000x reference)
"""Trainium2 Bass kernel for 3-layer GraphSAGE (nn_MCHCGraphSage).

Strategy (8 NeuronCores, SPMD single program):
  - Destination-sharded edges: core k owns dst nodes [k*6250, (k+1)*6250).
  - Features live in HBM as 256B rows addressed by "padded slab address"
    addr(n) = n + 22*(n//6250) (8 slabs of 6272 rows = 50176 rows), which
    makes the inter-layer AllGather output land directly in gather space.
  - Random x[src] rows are fetched with gpsimd dma_gather (int16 indices)
    round-robined over 4 SWDGE queues (disjoint Q7 core pairs overlap,
    ~3x the descriptor rate of one queue). int16 range forces a
    two-section split: section A gathers rows [0, 32768), section B rows
    [17408, 50176) (base offset 17408).
  - Segmented mean in ONE matmul level: edges are dst-sorted per window
    (no degree padding; only chunk-of-128 rounding). For each 128-slot
    chunk, PE accumulates meanT[64, 128dst] += gathered_chunk[128, 64].T
    @ oh[128, 128], where oh is a host-built scaled one-hot
    (value 1/deg at (slot, dst%128), zero for pad slots) streamed from
    HBM per batch.
  - Dense part per window, node-major: y = meanT.T @ Wl + hselfT.T @
    Ws_ext (bias folded as an extra ones-row of hselfT), ReLU on ACT,
    DMA the [128, 64] node-major block to the own slab; PE-transpose to
    keep the feature-major self slab for the next layer. AllGather
    between layers redistributes slabs.
"""

import os
import sys

import numpy as np

for _p in ("/opt/trn_rl_repo", "/root/.axon_site/_ro/trn_rl_repo"):
    if os.path.isdir(_p) and _p not in sys.path:
        sys.path.append(_p)

import ml_dtypes  # noqa: E402

N = 50000
D = 64
NCORES = 8
SLAB = 6250
PSLAB = 6272
WIN = 128
NW = PSLAB // WIN  # 49
TOTROW = NCORES * PSLAB  # 50176
# chunk-major gather address space: 7 cc-chunks of 7 windows each.
# addr(n) = j*7168 + k*896 + (l % 896) for node n = k*6250 + l, j = l//896.
# AllGather runs as 7 chunked collectives (one per j) that fire as soon as
# all cores have written those 7 windows, overlapping CC with layer tail.
CHW = 7               # windows per cc chunk
CH = CHW * WIN        # 896 rows per (core, chunk)
CSTRIDE = NCORES * CH  # 7168 rows per chunk in gather space
NCHK = NW // CHW      # 7 chunks
BBASE = 4 * CSTRIDE   # 28672: section A = chunks 0-3, B = chunks 4-6
BW = int(os.environ.get("SAGE_BW", "2"))  # windows per gather batch

_NC_CACHE = {}
LAST_RESULTS = None  # test harness introspection (exec_time_ns, profile)


def _addr(n):
    k, l = np.divmod(n, SLAB)
    return (l // CH) * CSTRIDE + k * CH + (l % CH)


def _pack(x, edge_index, scale):
    """Host-side packing. Returns per-core dicts + structure constants."""
    src = np.asarray(edge_index[0], dtype=np.int64)
    dst = np.asarray(edge_index[1], dtype=np.int64)
    addr_e = _addr(src)

    # pass 1: global chunk counts (structure shared across cores)
    nch_a = 0
    nch_b = 0
    per_core = []
    for k in range(NCORES):
        sel = (dst >= k * SLAB) & (dst < (k + 1) * SLAB)
        s_k = src[sel]
        d_k = dst[sel] - k * SLAB
        a_k = addr_e[sel]
        isA = a_k < BBASE
        degA = np.bincount(d_k[isA], minlength=PSLAB)
        degB = np.bincount(d_k[~isA], minlength=PSLAB)
        wA = degA.reshape(NW, WIN).sum(1).max()
        wB = degB.reshape(NW, WIN).sum(1).max()
        nch_a = max(nch_a, (int(wA) + 127) // 128)
        nch_b = max(nch_b, (int(wB) + 127) // 128)
        per_core.append((d_k, a_k, isA, degA, degB))

    S_A = nch_a * 128
    S_B = nch_b * 128
    NCH = nch_a + nch_b
    fdt = ml_dtypes.bfloat16

    # xext: node features in padded-slab address space, same for all cores
    xext = np.zeros((TOTROW, 128), dtype=fdt)
    rows = _addr(np.arange(N))
    xext[rows, :D] = x.astype(fdt)

    cores = []
    for k in range(NCORES):
        d_k, a_k, isA, degA, degB = per_core[k]

        def build(mask, deg, S, base, padval):
            """dst-sorted slot stream + (slot,dst) pairs for the one-hot."""
            e_d = d_k[mask]
            e_a = a_k[mask]
            order = np.argsort(e_d, kind="stable")
            d_s = e_d[order]
            a_s = e_a[order]
            off = (np.cumsum(deg.reshape(NW, WIN), 1) - deg.reshape(NW, WIN)
                   ).reshape(-1)
            start = np.concatenate([[0], np.cumsum(deg)])[:-1]
            rank = np.arange(len(d_s)) - start[d_s]
            pos = (d_s // WIN) * S + off[d_s] + rank  # slot within section
            stream = np.full(NW * S, padval, dtype=np.int64)
            stream[pos] = a_s - base
            return stream, pos, d_s

        # pad slots use idx 0: their one-hot columns are zero, so the
        # gathered value never contributes.
        streamA, posA, dA = build(isA, degA, S_A, 0, 0)
        streamB, posB, dB = build(~isA, degB, S_B, BBASE, 0)
        assert streamA.max() <= 32767 and streamB.max() <= 32767
        assert streamA.min() >= 0 and streamB.min() >= 0

        # scaled one-hot, laid out [128, NW * NCH * 128] bf16:
        # chunk index = w * NCH + (section chunk); col = chunk*128 + dst%128
        oh = np.zeros((128, NW * NCH * 128), dtype=fdt)
        for pos, d_s, sec0, S in ((posA, dA, 0, S_A), (posB, dB, nch_a, S_B)):
            w = pos // S
            sl = pos % S
            ch = w * NCH + sec0 + sl // 128
            row = sl % 128
            oh[row, ch * 128 + (d_s % WIN)] = scale[k * SLAB + d_s]

        stream = np.concatenate([streamA, streamB])  # A windows, then B
        idx16 = stream.astype(np.int16).reshape(-1, 16).T.copy()
        idx = np.tile(idx16, (8, 1))  # replicate for 8 gpsimd cores

        xselfT = np.zeros((D + 1, PSLAB), dtype=fdt)
        xselfT[:D, :SLAB] = x[k * SLAB : (k + 1) * SLAB].T.astype(fdt)
        xselfT[D, :] = 1.0  # bias row

        cores.append({"idx": idx, "onehot": oh, "xselfT": xselfT})

    return nch_a, nch_b, xext, cores


def _build_nc(nch_a, nch_b):
    import concourse.bacc as bacc
    import concourse.tile as tile
    import concourse.mybir as mybir

    dt = mybir.dt
    fdt = dt.bfloat16
    ROW = 128
    NCH = nch_a + nch_b
    S_A = nch_a * 128
    S_B = nch_b * 128
    SW = S_A + S_B  # idx slots per window (A block then B block)
    T = NW * SW

    nqueues = int(os.environ.get("SAGE_QUEUES", "4"))
    nc = bacc.Bacc(None, num_devices=NCORES, num_swdge_queues=nqueues)

    xext_d = nc.dram_tensor("xext", [TOTROW, ROW], fdt, kind="ExternalInput")
    idx_d = nc.dram_tensor("idx", [128, T // 16], dt.int16, kind="ExternalInput")
    oh_d = nc.dram_tensor(
        "onehot", [128, NW * NCH * 128], dt.bfloat16, kind="ExternalInput"
    )
    xsT_d = nc.dram_tensor("xselfT", [D + 1, PSLAB], fdt, kind="ExternalInput")
    ident_d = nc.dram_tensor("ident", [WIN, WIN], fdt, kind="ExternalInput")
    w_d = {}
    for l, m in ((0, D), (1, D), (2, 1)):
        w_d[f"wl{l}"] = nc.dram_tensor(f"wl{l}", [D, m], fdt, kind="ExternalInput")
        w_d[f"ws{l}"] = nc.dram_tensor(
            f"ws{l}", [D + 1, m], fdt, kind="ExternalInput"
        )
    out_d = nc.dram_tensor("out", [PSLAB, 1], dt.float32, kind="ExternalOutput")

    hext_ds = [
        nc.dram_tensor(f"hext{i}", [TOTROW, ROW], fdt, addr_space="Shared")
        for i in range(2)
    ]
    slab_ds = [nc.dram_tensor(f"slab{i}", [PSLAB, ROW], fdt) for i in range(2)]

    batches = []
    w0 = 0
    while w0 < NW:
        bw = min(BW, NW - w0)
        batches.append((w0, bw))
        w0 += bw
    n_layers = int(os.environ.get("SAGE_LAYERS", "3"))
    n_batch_lim = int(os.environ.get("SAGE_BATCHES", str(len(batches))))
    batches = batches[:n_batch_lim]
    no_cc = os.environ.get("SAGE_NOCC", "") == "1"
    gbufs = int(os.environ.get("SAGE_GBUFS", "6"))
    obufs = int(os.environ.get("SAGE_OBUFS", "3"))

    with tile.TileContext(nc) as tc:
        with (
            tc.tile_pool(name="const", bufs=1) as cpool,
            tc.tile_pool(name="gpool", bufs=gbufs) as gpool,
            tc.tile_pool(name="opool", bufs=obufs) as opool,
            tc.tile_pool(name="spool", bufs=3) as spool,
            tc.tile_pool(name="psB", bufs=3, space="PSUM") as psB,
            tc.tile_pool(name="psC", bufs=2, space="PSUM") as psC,
        ):
            idx_sb = cpool.tile([128, T // 16], dt.int16, tag="idx")
            ident_sb = cpool.tile([WIN, WIN], fdt, tag="ident")
            hs = [cpool.tile([D + 1, PSLAB], fdt, tag=f"hs{i}", name=f"hs{i}")
                  for i in range(3)]
            w_sb = {}
            for l, m in ((0, D), (1, D), (2, 1)):
                w_sb[f"wl{l}"] = cpool.tile([D, m], fdt, tag=f"wl{l}",
                                            name=f"wl{l}")
                w_sb[f"ws{l}"] = cpool.tile([D + 1, m], fdt, tag=f"ws{l}",
                                            name=f"ws{l}")
            zpad_sb = cpool.tile([PSLAB - SLAB, ROW], fdt, tag="zpad")

            nc.sync.dma_start(idx_sb[:], idx_d[:])
            nc.sync.dma_start(ident_sb[:], ident_d[:])
            nc.sync.dma_start(hs[0][:], xsT_d[:])
            for l in range(3):
                nc.sync.dma_start(w_sb[f"wl{l}"][:], w_d[f"wl{l}"][:])
                nc.sync.dma_start(w_sb[f"ws{l}"][:], w_d[f"ws{l}"][:])
            nc.vector.memset(zpad_sb[:], 0.0)
            nc.vector.memset(hs[1][D : D + 1, :], 1.0)
            nc.vector.memset(hs[2][D : D + 1, :], 1.0)

            import contextlib
            gq = [0]  # rotating queue counter for gather load balance
            reps = int(os.environ.get("SAGE_REPS", "1"))
            rep_cm = (tc.For_i(0, reps, 1, name="reploop")
                      if reps > 1 else contextlib.nullcontext())
            with rep_cm:
                for layer in range(n_layers):
                    src_t = xext_d if layer == 0 else hext_ds[(layer + 1) % 2]
                    slab_d = slab_ds[layer % 2]
                    hext_d = hext_ds[layer % 2]
                    hself = hs[layer]
                    wl_t = w_sb[f"wl{layer}"]
                    ws_t = w_sb[f"ws{layer}"]
                    m_out = 1 if layer == 2 else D

                    for bi, (w0, bw) in enumerate(batches):
                        gA = gpool.tile([128, bw * nch_a, ROW], fdt, tag="gA")
                        gB = gpool.tile([128, bw * nch_b, ROW], fdt, tag="gB")
                        oh_sb = opool.tile([128, bw * NCH * 128], dt.bfloat16,
                                           tag="ohs")
                        numA = bw * S_A
                        numB = bw * S_B
                        # idx stream layout: all A windows, then all B
                        a0 = w0 * S_A // 16
                        b0c = (NW * S_A + w0 * S_B) // 16
                        if os.environ.get("SAGE_NOOH", "") != "1":
                            nc.scalar.dma_start(
                                oh_sb[:],
                                oh_d[:, w0 * NCH * 128 : (w0 + bw) * NCH * 128],
                            )
                        nc.gpsimd.dma_gather(
                            gA[:], src_t[0:BBASE, :],
                            idx_sb[:, a0 : a0 + numA // 16],
                            numA, numA, ROW,
                            single_packet=False,
                            queue_num=gq[0] % nqueues,
                        )
                        nc.gpsimd.dma_gather(
                            gB[:], src_t[BBASE:, :],
                            idx_sb[:, b0c : b0c + numB // 16],
                            numB, numB, ROW,
                            single_packet=False,
                            queue_num=(gq[0] + 1) % nqueues,
                        )
                        gq[0] += 3

                        stage = int(os.environ.get("SAGE_STAGE", "9"))
                        for wi in range(bw):
                            if stage < 1:
                                break
                            w = w0 + wi
                            # meanT accumulation: one matmul per 128-slot chunk
                            win_ps = psB.tile([D, WIN], dt.float32, tag="winps")
                            for cc in range(NCH):
                                if cc < nch_a:
                                    lhs = gA[:, wi * nch_a + cc, 0:D]
                                else:
                                    lhs = gB[:, wi * nch_b + (cc - nch_a), 0:D]
                                oc = (wi * NCH + cc) * 128
                                nc.tensor.matmul(
                                    win_ps[:], lhs,
                                    oh_sb[:, oc : oc + 128],
                                    start=(cc == 0), stop=(cc == NCH - 1),
                                )
                            mean_sb = spool.tile([D, WIN], fdt, tag="mean")
                            nc.vector.tensor_copy(mean_sb[:], win_ps[:])
                            # dense, node-major: y = meanT.T@Wl + hselfT.T@Ws
                            y_ps = psC.tile([WIN, m_out], dt.float32, tag="ypsum")
                            nc.tensor.matmul(y_ps[:], mean_sb[:], wl_t[:],
                                             start=True, stop=False)
                            nc.tensor.matmul(y_ps[:],
                                             hself[:, w * WIN : (w + 1) * WIN],
                                             ws_t[:], start=False, stop=True)
                            if layer < 2:
                                hn_sb = spool.tile([WIN, D], fdt, tag="hn")
                                nc.scalar.activation(
                                    hn_sb[:], y_ps[:],
                                    mybir.ActivationFunctionType.Relu,
                                )
                                nc.sync.dma_start(
                                    slab_d[w * WIN : (w + 1) * WIN, 0:D], hn_sb[:]
                                )
                                t_ps = psB.tile([D, WIN], fdt, tag="tps",
                                                name="t_ps")
                                nc.tensor.transpose(t_ps[:], hn_sb[:], ident_sb[:])
                                nc.vector.tensor_copy(
                                    hs[layer + 1][0:D, w * WIN : (w + 1) * WIN],
                                    t_ps[:],
                                )
                            else:
                                y_sb = spool.tile([WIN, 1], dt.float32, tag="ysb")
                                nc.scalar.activation(
                                    y_sb[:], y_ps[:],
                                    mybir.ActivationFunctionType.Relu,
                                )
                                nc.sync.dma_start(
                                    out_d[w * WIN : (w + 1) * WIN, :], y_sb[:]
                                )

                        # fire each cc chunk's AllGather as soon as its 7
                        # windows are written, overlapping CC with the tail
                        if layer < 2 and layer < n_layers - 1 and not no_cc:
                            for j in range(NCHK):
                                if w0 <= CHW * j + CHW - 1 < w0 + bw:
                                    if j == NCHK - 1:
                                        nc.sync.dma_start(
                                            slab_d[SLAB:PSLAB, :], zpad_sb[:]
                                        )
                                    nc.gpsimd.collective_compute(
                                        "AllGather",
                                        mybir.AluOpType.bypass,
                                        replica_groups=[list(range(NCORES))],
                                        ins=[slab_d[j * CH : (j + 1) * CH, :]],
                                        outs=[hext_d[j * CSTRIDE :
                                                     (j + 1) * CSTRIDE, :]],
                                    )

    nc.compile()
    return nc


def kernel(**inputs):
    x = np.asarray(inputs["x"], dtype=np.float32)
    edge_index = np.asarray(inputs["edge_index"])

    deg = np.bincount(np.asarray(edge_index[1], dtype=np.int64), minlength=N)
    scale = np.where(deg > 0, 1.0 / np.maximum(deg, 1), 0.0).astype(np.float32)

    nch_a, nch_b, xext, cores = _pack(x, edge_index, scale)

    key = (nch_a, nch_b)
    if key not in _NC_CACHE:
        _NC_CACHE[key] = _build_nc(nch_a, nch_b)
    nc = _NC_CACHE[key]

    fdt = ml_dtypes.bfloat16
    ident = np.eye(WIN, dtype=fdt)

    common = {"xext": xext, "ident": ident}
    for l in range(3):
        common[f"wl{l}"] = np.asarray(inputs[f"Wl{l}"]).astype(fdt)
        wse = np.concatenate(
            [
                np.asarray(inputs[f"Ws{l}"], np.float32),
                (np.asarray(inputs[f"bl{l}"], np.float32)
                 + np.asarray(inputs[f"bs{l}"], np.float32)).reshape(1, -1),
            ],
            axis=0,
        )
        common[f"ws{l}"] = wse.astype(fdt)

    in_maps = []
    for k in range(NCORES):
        m = dict(common)
        m.update(cores[k])
        in_maps.append(m)

    from concourse.bass_utils import run_bass_kernel_spmd

    res = run_bass_kernel_spmd(nc, in_maps, core_ids=list(range(NCORES)))
    global LAST_RESULTS
    LAST_RESULTS = res
    outs = [np.asarray(res.results[k]["out"]).reshape(-1)[:SLAB]
            for k in range(NCORES)]
    return np.concatenate(outs).reshape(N, 1).astype(np.float32)


if __name__ == "__main__":
    pass
